# revision 42
# baseline (speedup 1.0000x reference)
"""Trainium2 Bass kernel for nn_BinarizeLayer.

out[b, f] = (medians[f] > 0) AND (inputs[b, f] >= medians[f])

Host preprocessing folds the two conditions into one comparison AND
prunes dead features: for the ~50% of features with medians[f] <= 0 the
output is False regardless of the input, so those input columns are
never shipped to (or read by) the device at all. The host gathers the
Fs = #(medians > 0) "live" columns into a compact [B, FP] array
(FP = Fs rounded up to even; multiple of 1024 in the rare FP > 4096
fallback), the device computes
out = x_gathered >= medians_gathered, and the host scatters the packed
result back into the full [B, 8192] output (False elsewhere). This
halves HBM read traffic, which is the roofline for this kernel.

Data-parallel over 8 NeuronCores, processed as chunks of 64 consecutive
rows (~1 MiB, fully contiguous in DRAM). The load's access pattern fans
each chunk onto 128 partitions: partition p holds half-row
(row 64i + p//2, cols (p%2)*CQ ..), keeping HBM reads sequential with
8 KiB descriptor lines -- the measured sweet spot (4 KiB lines net
~270 GB/s/core, 8 KiB ~300, 16 KiB drops DMA-engine concurrency and
nets ~240). All loads go down the single sync-ring HWDGE queue:
feeding a second hardware queue (scalar ring) or the gpsimd SWDGE
trips the chip's HAM governor, which slams a 4-of-8 DMA duty cycle on
most cores and nets ~190 GB/s. The sync ring carries ONLY load
dispatches: any dependent compute instruction interleaved into a
load-dispatch ring serializes the whole pipeline to one chunk in
flight (rings are in-order).

Per chunk: DVE compares against a median tile host-prepared in the same
per-partition layout, emitting 0/1 bf16 bits; the tensor engine
bit-packs 8 partitions per byte with one constant [128,16] matmul
weight (2^(p%8) block-diagonal), accumulating exact small integers in
PSUM. Four consecutive chunks accumulate into one [128, CQ] PSUM tile
(chunk sub's matmuls target the 16-partition stripe at base 32*sub;
PE tile_position allows output bases 0/32/64/96 only), so the
f32->u8 PSUM evacuation on the scalar engine runs once per 4 chunks:
a scalar activation instruction has ~1.4us fixed overhead and
per-partition cost, making a [128, CQ] copy no dearer than [16, CQ].
Each core stores ~1 MiB instead of ~8 MiB; the host unpacks bits and
inverts the partition bijection with pure reshapes.

Tail: every core's last chunk is loaded whole but compared/packed/
stored in <=512-column slabs on otherwise-idle engines (DVE evac,
sync-ring stores), so the post-load drain is ~2us instead of ~9us.

UNEVEN SHARDING: the HAM governor's hard 4/8-duty parking lands
almost exclusively on EVEN physical NeuronCores (= even logical
slots; the slot->core map [4,5,6,7,2,3,0,1] preserves parity), and
within the evens, slots 2/6 (nc6/nc0) are the chronic victims
(~0.24-0.25 chunks/us) while slots 0/4 (nc4/nc2) usually run free
(~0.29). Tiered counts 29 (slots 2,6) / 31 (slots 0,4) / 34 (odd
slots), sum 256 = full batch, emitted as one SPMD program with
If(pid%2) / If(pid!=2) / If(pid!=6) branches. Which core parks
hardest still drifts; the tiers bound the worst core near ~117us.
"""

import numpy as np

import concourse.bacc as bacc
import concourse.mybir as mybir
from concourse import tile
from concourse.bass_utils import run_bass_kernel_spmd

N_CORES = 8
B, F = 16384, 8192
P = 128  # SBUF partitions
G = P // 8  # packed bytes' groups per chunk (16)
MM_N = 512  # matmul free-dim chunk (one PSUM bank)
ROWS_PER_CORE = B // N_CORES  # 2048 (mean; see _counts shading)
# Chunk counts per logical slot for the standard k=32 geometry: odd
# slots park rarely (34); even slots 2 and 6 (physical nc6/nc0) are
# the chronic hard-park victims (29); even slots 0 and 4 (nc4/nc2)
# park occasionally (31). Sum = 256 chunks = full batch.
CNT32 = [31, 34, 29, 34, 31, 34, 29, 34]


def _counts(r):
    """Per-slot chunk counts and padded row capacity.

    Shading only for the standard c=2 geometry (k=32); the rare c=4
    fallback stays even.
    """
    k = ROWS_PER_CORE // r
    cnt = CNT32 if k == 32 else [k] * N_CORES
    return cnt, max(cnt) * r


def _geom(fs):
    """Padded width FP and chunk geometry for fs gathered columns.

    c = column segments per row (partition p holds segment p%c of row
    i*R + p//c); R = 128/c rows per chunk. c=2 gives ~8 KiB descriptor
    lines (the measured DMA sweet spot) while cq = FP/c fits a
    [128, cq] f32 PSUM tile in 4 banks (double-buffered); fall back to
    c=4 for FP > 4096. Matmul slab offsets (multiples of 512 f32) are
    PSUM-bank aligned for any cq.
    """
    if fs <= 4096:
        fp = max(64, -(-fs // 2) * 2)  # exact (even) width, c=2
        c = 2
    else:
        fp = -(-fs // 1024) * 1024
        c = 4
    r = P // c
    cq = fp // c
    return fp, c, r, cq


def _build(fp):
    """Build the SPMD program for FP = fp gathered (padded) columns."""
    fp, c, r, cq = _geom(fp)
    cnt, xrows = _counts(r)
    kmax = max(cnt)
    nc = bacc.Bacc(
        "TRN2",
        target_bir_lowering=False,
        debug=False,
        num_devices=N_CORES,
    )
    x = nc.declare_dram_parameter(
        "x", [xrows, fp], mybir.dt.float32, isOutput=False
    )
    med = nc.declare_dram_parameter(
        "med", [P // 4, cq], mybir.dt.float32, isOutput=False
    )
    pw = nc.declare_dram_parameter("pw", [P, G], mybir.dt.float32, isOutput=False)
    out = nc.declare_dram_parameter(
        "out", [kmax * G, cq], mybir.dt.uint8, isOutput=True
    )
    xv = x.rearrange("(i r) (c j) -> i (r c) j", r=r, c=c)

    # <=512-wide matmul slabs covering cq (last one may be partial).
    slabs = [(s, min(MM_N, cq - s)) for s in range(0, cq, MM_N)]
    # PSUM tiles are at most 2048 f32 wide (4 banks, per-partition);
    # a wider cq packs into n_ps half-width tiles, single-buffered
    # (their WAR recycle gap spans a whole chunk group, never stalls).
    w_ps = min(cq, 2048)
    n_ps = -(-cq // w_ps)
    ps_bufs = 2 if n_ps == 1 else 1
    assert n_ps == 1 or (cq % w_ps == 0 and w_ps % MM_N == 0)

    with tile.TileContext(nc) as tc:
        with (
            tc.tile_pool(name="const", bufs=1) as cpool,
            tc.tile_pool(name="xp", bufs=16) as xpool,
            tc.tile_pool(name="bp", bufs=5) as bpool,
            tc.tile_pool(name="op", bufs=3) as opool,
            tc.tile_pool(name="ps", bufs=2, space="PSUM") as pspool,
        ):
            # Constants on the scalar ring; the sync (load) ring is
            # purely x-loads from instruction 0. The median tile's
            # layout is periodic every c partitions; a 32-partition
            # replica is loaded and doubled twice on the DVE
            # (partition bases must be 32-aligned).
            med_t = cpool.tile([P, cq], mybir.dt.float32)
            nc.scalar.dma_start(out=med_t[0 : P // 4, :], in_=med[:])
            nc.vector.tensor_copy(
                out=med_t[P // 4 : P // 2, :], in_=med_t[0 : P // 4, :]
            )
            nc.vector.tensor_copy(
                out=med_t[P // 2 : P, :], in_=med_t[0 : P // 2, :]
            )
            pw_f32 = cpool.tile([P, G], mybir.dt.float32)
            pw_t = cpool.tile([P, G], mybir.dt.bfloat16)
            nc.scalar.dma_start(out=pw_f32[:], in_=pw[:])
            nc.vector.tensor_copy(out=pw_t[:], in_=pw_f32[:])

            def load(i):
                # Single HWDGE queue: ~275 GB/s/core. Dual-queue loads
                # (sync+scalar or sync+gpsimd) trip the chip's HAM
                # governor, which slams a 4/8 DMA duty cycle on most
                # cores for most of the run and nets ~190 GB/s.
                xt = xpool.tile([P, cq], mybir.dt.float32, tag="x")
                nc.sync.dma_start(out=xt[:], in_=xv[i][:])
                return xt

            # GROUP consecutive chunks accumulate into wide PSUM
            # tiles: chunk sub's pack-matmuls target the 16-partition
            # stripe at base 32*sub (PE tile_position only allows
            # output partition bases 0/32/64/96). One scalar
            # evacuation per PSUM tile then covers GROUP chunks (a
            # scalar activation copy has ~1.4us fixed overhead, and
            # its per-partition cost makes [16,w] cost the same as
            # [128,w] -- evacuate wide, rarely).
            GROUP = 4

            def ps_tiles():
                return [
                    pspool.tile(
                        [P, w_ps],
                        mybir.dt.float32,
                        tag=f"ps{h}",
                        name=f"ps{h}",
                        bufs=ps_bufs,
                    )
                    for h in range(n_ps)
                ]

            def emit_group(i0, n):
                ps = ps_tiles()
                for sub in range(n):
                    xt = load(i0 + sub)
                    bt = bpool.tile([P, cq], mybir.dt.bfloat16, tag="b")
                    nc.vector.tensor_tensor(
                        bt[:], xt[:], med_t[:], mybir.AluOpType.is_ge
                    )
                    p0 = 32 * sub
                    for s, w in slabs:
                        nc.tensor.matmul(
                            ps[s // w_ps][p0 : p0 + G, s % w_ps : s % w_ps + w],
                            pw_t[:],
                            bt[:, s : s + w],
                            start=True,
                            stop=True,
                            tile_position=(0, p0),
                        )
                for h in range(n_ps):
                    pk = opool.tile([P, w_ps], mybir.dt.uint8, tag="o")
                    nc.scalar.copy(out=pk[:], in_=ps[h][:])
                    for s2 in range(n):
                        nc.scalar.dma_start(
                            out=out[
                                (i0 + s2) * G : (i0 + s2 + 1) * G,
                                h * w_ps : (h + 1) * w_ps,
                            ],
                            in_=pk[32 * s2 : 32 * s2 + G, :],
                        )

            def slab_tail(i):
                # Last chunk: compare/pack/evacuate/store per <=512-col
                # slab on otherwise-idle engines, so the post-load
                # drain is ~2us instead of ~9us.
                xt = load(i)
                ps = ps_tiles()
                for s, w in slabs:
                    bt = bpool.tile([P, w], mybir.dt.bfloat16, tag="bt")
                    nc.vector.tensor_tensor(
                        bt[:],
                        xt[:, s : s + w],
                        med_t[:, s : s + w],
                        mybir.AluOpType.is_ge,
                    )
                    psh = ps[s // w_ps]
                    sl = s % w_ps
                    nc.tensor.matmul(
                        psh[:G, sl : sl + w],
                        pw_t[:],
                        bt[:],
                        start=True,
                        stop=True,
                        tile_position=(0, 0),
                    )
                    pk = opool.tile([G, w], mybir.dt.uint8, tag="ot")
                    nc.vector.tensor_copy(out=pk[:], in_=psh[:G, sl : sl + w])
                    nc.sync.dma_start(
                        out=out[i * G : (i + 1) * G, s : s + w], in_=pk[:]
                    )

            if min(cnt) == max(cnt):
                # Even split: common groups, 3-chunk group, slab tail.
                k = cnt[0]
                assert k % GROUP == 0, (k, GROUP)
                for gi in range(k // GROUP - 1):
                    emit_group(gi * GROUP, GROUP)
                emit_group(k - GROUP, GROUP - 1)
                slab_tail(k - 1)
            else:
                # Tiered shading (29/31/34): chunks 0..27 in full
                # groups for everyone; odd slots then run one more
                # full group + one chunk + slab tail (34), even slots
                # 0/4 two chunks + slab tail (31), even slots 2/6 the
                # slab tail alone (29).
                assert cnt == CNT32 and min(cnt) == 29, cnt
                for gi in range(7):
                    emit_group(gi * GROUP, GROUP)
                pid = nc.partition_id()
                with tc.If(pid % 2 == 1) as codd:
                    emit_group(28, GROUP)
                    emit_group(32, 1)
                    slab_tail(33)
                with codd.Else():
                    with tc.If(pid != 2) as c2:
                        with tc.If(pid != 6) as c6:
                            emit_group(28, 2)
                            slab_tail(30)
                        with c6.Else():
                            slab_tail(28)
                    with c2.Else():
                        slab_tail(28)
    nc.compile()
    return nc


def _pack_weights():
    pw = np.zeros((P, G), dtype=np.float32)
    for p in range(P):
        pw[p, p // 8] = float(1 << (p % 8))
    return pw


def _select(medians):
    """Live-feature index set and padded width FP."""
    m = np.asarray(medians, dtype=np.float32)
    idx = np.flatnonzero(m > 0)
    fs = int(idx.size)
    fp, _, _, _ = _geom(max(fs, 1))
    return m, idx, fs, fp


def _in_maps(inputs, medians):
    x = np.asarray(inputs, dtype=np.float32)
    m, idx, fs, fp = _select(medians)
    fp, c, r, cq = _geom(fs)
    # Gathered medians, padded with +inf (pad columns compare False).
    m2 = np.full(fp, np.inf, dtype=np.float32)
    m2[:fs] = m[idx]
    med = np.ascontiguousarray(
        np.broadcast_to(
            m2.reshape(1, c, cq), (P // 4 // c, c, cq)
        ).reshape(P // 4, cq)
    )
    pw = _pack_weights()
    xg = x[:, idx]  # [B, fs] gathered live columns
    cnt, xrows = _counts(r)
    rows_per = [cnt[ci] * r for ci in range(N_CORES)]
    starts = np.concatenate([[0], np.cumsum(rows_per)])
    assert starts[-1] == B, starts
    maps = []
    for ci in range(N_CORES):
        xc = np.zeros((xrows, fp), dtype=np.float32)
        xc[: rows_per[ci], :fs] = xg[starts[ci] : starts[ci] + rows_per[ci]]
        maps.append({"x": xc, "med": med, "pw": pw})
    return maps


def _decode(packed, fp, ci):
    """[kmax*G, cq] u8 -> [rows_for_core_ci, fp] 0/1 rows."""
    fp, c, r, cq = _geom(fp)
    cnt, xrows = _counts(r)
    k = cnt[ci]
    a = packed[: k * G].reshape(k, G, 1, cq)
    bits = np.unpackbits(a, axis=2, bitorder="little")  # [i, g, kbit, j]
    # partition p = 8g + kbit -> (row p//c, segment p%c)
    bits = bits.reshape(k, P, cq).reshape(k, r, c, cq)
    return bits.reshape(k * r, fp)


def kernel(inputs, medians):
    m, idx, fs, fp = _select(medians)
    if fs == 0:
        return np.zeros((np.asarray(inputs).shape[0], m.size), dtype=bool)
    in_maps = _in_maps(inputs, medians)
    last_err = None
    for _ in range(3):  # transient axon/NRT failures happen; retry
        try:
            nc = _build(fs)
            res = run_bass_kernel_spmd(nc, in_maps, list(range(N_CORES))).results
            break
        except Exception as e:  # noqa: BLE001
            last_err = e
    else:
        raise last_err
    gathered = np.concatenate(
        [_decode(r["out"], fs, ci) for ci, r in enumerate(res)], axis=0
    )
    out = np.zeros((gathered.shape[0], m.size), dtype=bool)
    out[:, idx] = gathered[:, :fs].astype(bool)
    return out


# revision 43
# speedup vs baseline: 1.0203x; 1.0203x over previous
"""Trainium2 Bass kernel for nn_BinarizeLayer.

out[b, f] = (medians[f] > 0) AND (inputs[b, f] >= medians[f])

Host preprocessing folds the two conditions into one comparison AND
prunes dead features: for the ~50% of features with medians[f] <= 0 the
output is False regardless of the input, so those input columns are
never shipped to (or read by) the device at all. The host gathers the
Fs = #(medians > 0) "live" columns into a compact [B, FP] array
(FP = Fs rounded up to even; multiple of 1024 in the rare FP > 4096
fallback), the device computes
out = x_gathered >= medians_gathered, and the host scatters the packed
result back into the full [B, 8192] output (False elsewhere). This
halves HBM read traffic, which is the roofline for this kernel.

Data-parallel over 8 NeuronCores, processed as chunks of 64 consecutive
rows (~1 MiB, fully contiguous in DRAM). The load's access pattern fans
each chunk onto 128 partitions: partition p holds half-row
(row 64i + p//2, cols (p%2)*CQ ..), keeping HBM reads sequential with
8 KiB descriptor lines -- the measured sweet spot (4 KiB lines net
~270 GB/s/core, 8 KiB ~300, 16 KiB drops DMA-engine concurrency and
nets ~240). All loads go down the single sync-ring HWDGE queue:
feeding a second hardware queue (scalar ring) or the gpsimd SWDGE
trips the chip's HAM governor, which slams a 4-of-8 DMA duty cycle on
most cores and nets ~190 GB/s. The sync ring carries ONLY load
dispatches: any dependent compute instruction interleaved into a
load-dispatch ring serializes the whole pipeline to one chunk in
flight (rings are in-order).

Per chunk: DVE compares against a median tile host-prepared in the same
per-partition layout, emitting 0/1 bf16 bits; the tensor engine
bit-packs 8 partitions per byte with one constant [128,16] matmul
weight (2^(p%8) block-diagonal), accumulating exact small integers in
PSUM. Four consecutive chunks accumulate into one [128, CQ] PSUM tile
(chunk sub's matmuls target the 16-partition stripe at base 32*sub;
PE tile_position allows output bases 0/32/64/96 only), so the
f32->u8 PSUM evacuation on the scalar engine runs once per 4 chunks:
a scalar activation instruction has ~1.4us fixed overhead and
per-partition cost, making a [128, CQ] copy no dearer than [16, CQ].
Each core stores ~1 MiB instead of ~8 MiB; the host unpacks bits and
inverts the partition bijection with pure reshapes.

Tail: every core's last chunk is loaded whole but compared/packed/
stored in <=512-column slabs on otherwise-idle engines (DVE evac,
sync-ring stores), so the post-load drain is ~2us instead of ~9us.

UNEVEN SHARDING: the HAM governor's hard 4/8-duty parking lands
almost exclusively on EVEN physical NeuronCores (= even logical
slots; the slot->core map [4,5,6,7,2,3,0,1] preserves parity).
Across profiled runs even cores net ~0.25 chunks/us vs ~0.30 for odd
cores, so even slots get 30 chunks and odd slots 34 (sum 256 = full
batch), emitted as one SPMD program with an If(pid % 2) branch.
Which even core parks hardest still drifts run to run; the parity
split bounds the worst core near ~115-120us either way.
"""

import numpy as np

import concourse.bacc as bacc
import concourse.mybir as mybir
from concourse import tile
from concourse.bass_utils import run_bass_kernel_spmd

N_CORES = 8
B, F = 16384, 8192
P = 128  # SBUF partitions
G = P // 8  # packed bytes' groups per chunk (16)
MM_N = 512  # matmul free-dim chunk (one PSUM bank)
ROWS_PER_CORE = B // N_CORES  # 2048 (mean; see _counts parity shading)
SHADE = 2  # chunks shifted from each even slot to its odd neighbor


def _counts(r):
    """Per-slot chunk counts [even, odd] and padded row capacity.

    Parity shading only for the standard c=2 geometry (k=32); the
    rare c=4 fallback stays even.
    """
    k = ROWS_PER_CORE // r
    if k == 32:
        ke, ko = k - SHADE, k + SHADE
    else:
        ke = ko = k
    return ke, ko, max(ke, ko) * r


def _geom(fs):
    """Padded width FP and chunk geometry for fs gathered columns.

    c = column segments per row (partition p holds segment p%c of row
    i*R + p//c); R = 128/c rows per chunk. c=2 gives ~8 KiB descriptor
    lines (the measured DMA sweet spot) while cq = FP/c fits a
    [128, cq] f32 PSUM tile in 4 banks (double-buffered); fall back to
    c=4 for FP > 4096. Matmul slab offsets (multiples of 512 f32) are
    PSUM-bank aligned for any cq.
    """
    if fs <= 4096:
        fp = max(64, -(-fs // 2) * 2)  # exact (even) width, c=2
        c = 2
    else:
        fp = -(-fs // 1024) * 1024
        c = 4
    r = P // c
    cq = fp // c
    return fp, c, r, cq


def _build(fp):
    """Build the SPMD program for FP = fp gathered (padded) columns."""
    fp, c, r, cq = _geom(fp)
    ke, ko, xrows = _counts(r)
    nc = bacc.Bacc(
        "TRN2",
        target_bir_lowering=False,
        debug=False,
        num_devices=N_CORES,
    )
    x = nc.declare_dram_parameter(
        "x", [xrows, fp], mybir.dt.float32, isOutput=False
    )
    med = nc.declare_dram_parameter(
        "med", [P // 4, cq], mybir.dt.float32, isOutput=False
    )
    pw = nc.declare_dram_parameter("pw", [P, G], mybir.dt.float32, isOutput=False)
    out = nc.declare_dram_parameter(
        "out", [ko * G, cq], mybir.dt.uint8, isOutput=True
    )
    xv = x.rearrange("(i r) (c j) -> i (r c) j", r=r, c=c)

    # <=512-wide matmul slabs covering cq (last one may be partial).
    slabs = [(s, min(MM_N, cq - s)) for s in range(0, cq, MM_N)]
    # PSUM tiles are at most 2048 f32 wide (4 banks, per-partition);
    # a wider cq packs into n_ps half-width tiles, single-buffered
    # (their WAR recycle gap spans a whole chunk group, never stalls).
    w_ps = min(cq, 2048)
    n_ps = -(-cq // w_ps)
    ps_bufs = 2 if n_ps == 1 else 1
    assert n_ps == 1 or (cq % w_ps == 0 and w_ps % MM_N == 0)

    with tile.TileContext(nc) as tc:
        with (
            tc.tile_pool(name="const", bufs=1) as cpool,
            tc.tile_pool(name="xp", bufs=16) as xpool,
            tc.tile_pool(name="bp", bufs=5) as bpool,
            tc.tile_pool(name="op", bufs=3) as opool,
            tc.tile_pool(name="ps", bufs=2, space="PSUM") as pspool,
        ):
            # Constants on the scalar ring; the sync (load) ring is
            # purely x-loads from instruction 0. The median tile's
            # layout is periodic every c partitions; a 32-partition
            # replica is loaded and doubled twice on the DVE
            # (partition bases must be 32-aligned).
            med_t = cpool.tile([P, cq], mybir.dt.float32)
            nc.scalar.dma_start(out=med_t[0 : P // 4, :], in_=med[:])
            nc.vector.tensor_copy(
                out=med_t[P // 4 : P // 2, :], in_=med_t[0 : P // 4, :]
            )
            nc.vector.tensor_copy(
                out=med_t[P // 2 : P, :], in_=med_t[0 : P // 2, :]
            )
            pw_f32 = cpool.tile([P, G], mybir.dt.float32)
            pw_t = cpool.tile([P, G], mybir.dt.bfloat16)
            nc.scalar.dma_start(out=pw_f32[:], in_=pw[:])
            nc.vector.tensor_copy(out=pw_t[:], in_=pw_f32[:])

            def load(i):
                # Single HWDGE queue: ~275 GB/s/core. Dual-queue loads
                # (sync+scalar or sync+gpsimd) trip the chip's HAM
                # governor, which slams a 4/8 DMA duty cycle on most
                # cores for most of the run and nets ~190 GB/s.
                xt = xpool.tile([P, cq], mybir.dt.float32, tag="x")
                nc.sync.dma_start(out=xt[:], in_=xv[i][:])
                return xt

            # GROUP consecutive chunks accumulate into wide PSUM
            # tiles: chunk sub's pack-matmuls target the 16-partition
            # stripe at base 32*sub (PE tile_position only allows
            # output partition bases 0/32/64/96). One scalar
            # evacuation per PSUM tile then covers GROUP chunks (a
            # scalar activation copy has ~1.4us fixed overhead, and
            # its per-partition cost makes [16,w] cost the same as
            # [128,w] -- evacuate wide, rarely).
            GROUP = 4

            def ps_tiles():
                return [
                    pspool.tile(
                        [P, w_ps],
                        mybir.dt.float32,
                        tag=f"ps{h}",
                        name=f"ps{h}",
                        bufs=ps_bufs,
                    )
                    for h in range(n_ps)
                ]

            def emit_group(i0, n):
                ps = ps_tiles()
                for sub in range(n):
                    xt = load(i0 + sub)
                    bt = bpool.tile([P, cq], mybir.dt.bfloat16, tag="b")
                    nc.vector.tensor_tensor(
                        bt[:], xt[:], med_t[:], mybir.AluOpType.is_ge
                    )
                    p0 = 32 * sub
                    for s, w in slabs:
                        nc.tensor.matmul(
                            ps[s // w_ps][p0 : p0 + G, s % w_ps : s % w_ps + w],
                            pw_t[:],
                            bt[:, s : s + w],
                            start=True,
                            stop=True,
                            tile_position=(0, p0),
                        )
                for h in range(n_ps):
                    pk = opool.tile([P, w_ps], mybir.dt.uint8, tag="o")
                    nc.scalar.copy(out=pk[:], in_=ps[h][:])
                    for s2 in range(n):
                        nc.scalar.dma_start(
                            out=out[
                                (i0 + s2) * G : (i0 + s2 + 1) * G,
                                h * w_ps : (h + 1) * w_ps,
                            ],
                            in_=pk[32 * s2 : 32 * s2 + G, :],
                        )

            def slab_tail(i):
                # Last chunk: compare/pack/evacuate/store per <=512-col
                # slab on otherwise-idle engines, so the post-load
                # drain is ~2us instead of ~9us.
                xt = load(i)
                ps = ps_tiles()
                for s, w in slabs:
                    bt = bpool.tile([P, w], mybir.dt.bfloat16, tag="bt")
                    nc.vector.tensor_tensor(
                        bt[:],
                        xt[:, s : s + w],
                        med_t[:, s : s + w],
                        mybir.AluOpType.is_ge,
                    )
                    psh = ps[s // w_ps]
                    sl = s % w_ps
                    nc.tensor.matmul(
                        psh[:G, sl : sl + w],
                        pw_t[:],
                        bt[:],
                        start=True,
                        stop=True,
                        tile_position=(0, 0),
                    )
                    pk = opool.tile([G, w], mybir.dt.uint8, tag="ot")
                    nc.vector.tensor_copy(out=pk[:], in_=psh[:G, sl : sl + w])
                    nc.sync.dma_start(
                        out=out[i * G : (i + 1) * G, s : s + w], in_=pk[:]
                    )

            if ke == ko:
                # Even split: common groups, 3-chunk group, slab tail.
                k = ke
                assert k % GROUP == 0, (k, GROUP)
                for gi in range(k // GROUP - 1):
                    emit_group(gi * GROUP, GROUP)
                emit_group(k - GROUP, GROUP - 1)
                slab_tail(k - 1)
            else:
                # Parity shading (ke=30, ko=34): chunks 0..27 in full
                # groups for everyone; even slots then run one chunk +
                # slab tail, odd slots one more full group, one chunk,
                # and the slab tail.
                assert ke == 30 and ko == 34, (ke, ko)
                for gi in range(7):
                    emit_group(gi * GROUP, GROUP)
                pid = nc.partition_id()
                with tc.If(pid % 2 == 1) as codd:
                    emit_group(28, GROUP)
                    emit_group(32, 1)
                    slab_tail(33)
                with codd.Else():
                    emit_group(28, 1)
                    slab_tail(29)
    nc.compile()
    return nc


def _pack_weights():
    pw = np.zeros((P, G), dtype=np.float32)
    for p in range(P):
        pw[p, p // 8] = float(1 << (p % 8))
    return pw


def _select(medians):
    """Live-feature index set and padded width FP."""
    m = np.asarray(medians, dtype=np.float32)
    idx = np.flatnonzero(m > 0)
    fs = int(idx.size)
    fp, _, _, _ = _geom(max(fs, 1))
    return m, idx, fs, fp


def _in_maps(inputs, medians):
    x = np.asarray(inputs, dtype=np.float32)
    m, idx, fs, fp = _select(medians)
    fp, c, r, cq = _geom(fs)
    # Gathered medians, padded with +inf (pad columns compare False).
    m2 = np.full(fp, np.inf, dtype=np.float32)
    m2[:fs] = m[idx]
    med = np.ascontiguousarray(
        np.broadcast_to(
            m2.reshape(1, c, cq), (P // 4 // c, c, cq)
        ).reshape(P // 4, cq)
    )
    pw = _pack_weights()
    xg = x[:, idx]  # [B, fs] gathered live columns
    ke, ko, xrows = _counts(r)
    rows_per = [(ko if ci % 2 else ke) * r for ci in range(N_CORES)]
    starts = np.concatenate([[0], np.cumsum(rows_per)])
    assert starts[-1] == B, starts
    maps = []
    for ci in range(N_CORES):
        xc = np.zeros((xrows, fp), dtype=np.float32)
        xc[: rows_per[ci], :fs] = xg[starts[ci] : starts[ci] + rows_per[ci]]
        maps.append({"x": xc, "med": med, "pw": pw})
    return maps


def _decode(packed, fp, ci):
    """[ko*G, cq] u8 -> [rows_for_core_ci, fp] 0/1 rows."""
    fp, c, r, cq = _geom(fp)
    ke, ko, xrows = _counts(r)
    k = ko if ci % 2 else ke
    a = packed[: k * G].reshape(k, G, 1, cq)
    bits = np.unpackbits(a, axis=2, bitorder="little")  # [i, g, kbit, j]
    # partition p = 8g + kbit -> (row p//c, segment p%c)
    bits = bits.reshape(k, P, cq).reshape(k, r, c, cq)
    return bits.reshape(k * r, fp)


def kernel(inputs, medians):
    m, idx, fs, fp = _select(medians)
    if fs == 0:
        return np.zeros((np.asarray(inputs).shape[0], m.size), dtype=bool)
    in_maps = _in_maps(inputs, medians)
    last_err = None
    for _ in range(3):  # transient axon/NRT failures happen; retry
        try:
            nc = _build(fs)
            res = run_bass_kernel_spmd(nc, in_maps, list(range(N_CORES))).results
            break
        except Exception as e:  # noqa: BLE001
            last_err = e
    else:
        raise last_err
    gathered = np.concatenate(
        [_decode(r["out"], fs, ci) for ci, r in enumerate(res)], axis=0
    )
    out = np.zeros((gathered.shape[0], m.size), dtype=bool)
    out[:, idx] = gathered[:, :fs].astype(bool)
    return out
